# revision 36
# baseline (speedup 1.0000x reference)
"""BoxTightnessPriorLoss Trainium2 kernel (v3: cropped separable DoubleRow).

Inputs (full, host-side):
  logits:    (2, 4, 128, 128, 128) float32   -- (B, C, W, H, D)
  box_masks: (2, 4, 4, 128, 128, 128) bool   -- (B, C, N, W, H, D)

Math: every box mask is a product of three interval indicators
mask[n,w,h,d] = mw[n,w]*mh[n,h]*md[n,d], so each slice profile is a
bilinear contraction of logits with two marginals:
  T_d[n,d] = sum_{w,h} mw mh L      (then sl_d = md * T_d)
  U[n,h]   = sum_{w,d} mw md L      (then sl_h = mh * U)
  V[n,w]   = sum_{h,d} mh md L      (then sl_w = mw * V)
Marginals are exact via stride-16 subsampling (box sides >= 16).

Only logits rows/planes inside the per-axis box-marginal UNION contribute
(all G entries outside are zero), so each core gathers its union indices
(padded with complement indices, whose marginals are zero, to the fixed
crop size S) and the device streams two fp8 layouts of the S^3 crop:
  lg[w', hpos*S + d']   and   lt[d', hpos*S + w']
with h-planes permuted so each 8S-col block q holds planes h = 8q+2j+kt
at position (kt, j), making the T/V passes clean 3D [part, kt, f]
DoubleRow fp8 patterns and U a 4D variant with (h, j) minor order.
Host extracts j-diagonal blocks, scatters through the index selections,
and finishes the tiny per-(b,c,n) segment math in float32.
"""
import os
import numpy as np

B, C, N, DM = 2, 4, 4, 128
SEG_W = 8
N_SEG = DM // SEG_W  # 16
N_CORES = 8

S_CROP = int(os.environ.get("BOXLOSS_S", "104"))  # crop size: multiple
# of 8, >= max per-axis box-union size across cores (seed-0 max is 101;
# kernel() falls back to the full S=128 program if any union overflows).

# U-pass moving AP is 4D; set 0 for the 3D no-j-block fallback (sim-able).
U_4D = bool(int(os.environ.get("BOXLOSS_U4D", "1")))

_compiled = {}  # S -> nc


def _hperm(S):
    p = np.arange(S)
    return (8 * (p >> 3) + 2 * (p & 3) + ((p >> 2) & 1)).astype(np.int64)


def _install_wait_split_patch():
    """This container's walrus (CoreV3) allows only ONE sync-wait per
    instruction; TileContext can attach several.  Split any instruction
    carrying N>1 waits into N-1 preceding wait-only NoOps (same engine)."""
    import concourse.tile as _tile
    import concourse.mybir as _mybir

    if getattr(_tile.TileContext, "_ant_wait_split", False):
        return
    _orig = _tile.TileContext.schedule_and_allocate

    def _split_multi_waits(nc):
        for func in nc.m.functions:
            for bb in func.blocks:
                insts = bb.instructions
                i = 0
                while i < len(insts):
                    inst = insts[i]
                    si = getattr(inst, "sync_info", None)
                    if si is not None and si.on_wait and len(si.on_wait) > 1:
                        waits = list(si.on_wait)
                        si.on_wait = [waits[-1]]
                        nops = []
                        for w in waits[:-1]:
                            nop = _mybir.InstNoOp(
                                name=nc.get_next_instruction_name(),
                                engine=inst.engine,
                                sync_info=_mybir.SyncInfo(on_wait=[w], on_update=[]),
                                bass_nofuse=True,
                            )
                            nops.append(nop)
                            nc.register_instruction(nop, overwrite=True)
                        insts[i:i] = nops
                        i += len(nops)
                    i += 1

    def _patched(self, *a, **kw):
        ret = _orig(self, *a, **kw)
        _split_multi_waits(self.nc)
        return ret

    _tile.TileContext.schedule_and_allocate = _patched
    _tile.TileContext._ant_wait_split = True


def _build(S):
    import concourse.bass as bass
    import concourse.tile as tile
    from concourse import mybir

    _install_wait_split_patch()

    f32 = mybir.dt.float32
    bf16 = mybir.dt.bfloat16
    f8 = mybir.dt.float8e4
    wide = mybir.dt.uint64  # DMA APs bitcast to wide elements
    DR = mybir.MatmulPerfMode.DoubleRow

    NQ = S // 8         # q-blocks per pass
    BL = 8 * S          # cols per q-block
    F = 4 * S           # out free size per pass
    COLS = S * S

    nc = bass.Bass()
    # single concatenated input: [G (3F cols) | lt (COLS) | lg (COLS)]
    GOFF, LTOFF, LGOFF = 0, 3 * F, 3 * F + COLS
    TOT = 3 * F + 2 * COLS
    ain = nc.dram_tensor("ain", [S, TOT], f8, kind="ExternalInput")
    o_all = nc.dram_tensor("o_all", [16, 3 * F], f32, kind="ExternalOutput")

    # stream chunks ~0.5 MiB: readiness latency scales with chunk size,
    # while per-dma_start issue cost is ~0.8us -- this balances both.
    edges = [0, LTOFF + 5 * BL, LTOFF + 10 * BL, LGOFF,
             LGOFF + 5 * BL, LGOFF + 10 * BL, LGOFF + (NQ - 1) * BL, TOT]

    with tile.TileContext(nc) as tc:
        with (
            tc.tile_pool(name="consts", bufs=1) as consts,
            tc.tile_pool(name="big", bufs=1) as big,
            tc.tile_pool(name="outs", bufs=1) as outs,
            tc.tile_pool(name="wpsum", bufs=2, space="PSUM") as wpsum,
            tc.tile_pool(name="apsum", bufs=1, space="PSUM") as apsum,
        ):
            # warm-up source tile; contents are irrelevant (never read back)
            warm = consts.tile([DM, 512], bf16)
            nc.vector.memset(warm[:], 0.0)

            ALL = big.tile([S, TOT], f8)

            # ---- input stream, one HWDGE ring (sync), FIFO chunks
            for lo, hi in zip(edges[:-1], edges[1:]):
                nc.sync.dma_start(out=ALL[:, lo:hi].bitcast(wide),
                                  in_=ain[:, lo:hi].bitcast(wide))

            # ---- PE clock warm-up while DMAs stream
            for i in range(12):
                wp = wpsum.tile([1, 512], f32, tag="wp")
                nc.tensor.matmul(wp[:], warm[:, :1], warm[:], start=True, stop=True)

            p_t = apsum.tile([16, F], f32)
            p_u = apsum.tile([16, F], f32)
            p_v = apsum.tile([16, F], f32)

            def dr3(pout, gcol, loff, q, start, stop):
                nc.tensor.matmul(
                    pout,
                    ALL[:, gcol + 32 * q:gcol + 32 * (q + 1)].rearrange(
                        "p (kt m) -> p kt m", kt=2),
                    ALL[:, loff + BL * q:loff + BL * (q + 1)].rearrange(
                        "p (kt f) -> p kt f", kt=2),
                    start=start, stop=stop, perf_mode=DR,
                )

            # ---- V pass (lt, pipelined on its chunks)
            for q in range(NQ):
                dr3(p_v[:], 2 * F, LTOFF, q, q == 0, q == NQ - 1)

            # ---- U pass (lt, needs full lt); f-order (h, j) keeps the
            # moving feed in 8-byte contiguous runs.
            if U_4D:
                Lt_u = ALL[:, LTOFF:LTOFF + COLS].rearrange(
                    "p (h qq j kt) -> p qq kt h j", qq=NQ, j=4, kt=2)
                for q in range(NQ):
                    nc.tensor.matmul(
                        p_u[:],
                        ALL[:, F + 32 * q:F + 32 * (q + 1)].rearrange(
                            "p (kt m) -> p kt m", kt=2),
                        Lt_u[:, q],
                        start=(q == 0), stop=(q == NQ - 1), perf_mode=DR,
                    )
            else:
                Lt_u3 = ALL[:, LTOFF:LTOFF + COLS].rearrange(
                    "p (h s kt) -> p s kt h", s=S // 2, kt=2)
                for s in range(S // 2):
                    nc.tensor.matmul(
                        p_u[:4, :S],
                        ALL[:, F + 8 * s:F + 8 * (s + 1)].rearrange(
                            "p (kt m) -> p kt m", kt=2),
                        Lt_u3[:, s],
                        start=(s == 0), stop=(s == S // 2 - 1), perf_mode=DR,
                    )

            # ---- T pass (lg, pipelined on its chunks)
            for q in range(NQ):
                dr3(p_t[:], 0, LGOFF, q, q == 0, q == NQ - 1)

            stage = outs.tile([16, 3 * F], f32)
            nc.vector.tensor_copy(stage[:, 2 * F:3 * F], p_v[:])
            if U_4D:
                nc.vector.tensor_copy(stage[:, F:2 * F], p_u[:])
            else:
                nc.vector.memset(stage[:, F:2 * F], 0.0)
                nc.vector.tensor_copy(stage[:4, F:F + S], p_u[:4, :S])
            # ship U+V as soon as staged; T separately at the end
            nc.sync.dma_start(out=o_all[:, F:3 * F], in_=stage[:, F:3 * F])
            nc.vector.tensor_copy(stage[:, 0:F], p_t[:])
            nc.sync.dma_start(out=o_all[:, 0:F], in_=stage[:, 0:F])

    return nc


def _host_marginals(box_masks):
    """Exact interval marginals via stride-16 subsampling (sides >= 16)."""
    mw = box_masks[:, :, :, :, ::16, ::16].any(axis=(4, 5))  # (B,C,N,W)
    mh = box_masks[:, :, :, ::16, :, ::16].any(axis=(3, 5))  # (B,C,N,H)
    md = box_masks[:, :, :, ::16, ::16, :].any(axis=(3, 4))  # (B,C,N,D)
    return mw, mh, md


def _sel_pad(m, S):
    """Union indices of (N, 128) marginal, padded to S with complement."""
    u = m.any(axis=0)
    idx = np.nonzero(u)[0]
    if len(idx) > S:
        return None
    comp = np.nonzero(~u)[0]
    return np.concatenate([idx, comp[:S - len(idx)]])


def _build_g(a, b_sel):
    """G[part, q, kt, j, n] = a[n, part] * b_sel[n, q, kt, j] -> (S, 4S)."""
    g = np.einsum('np,nqkj->pqkjn', a.astype(np.float32),
                  b_sel.astype(np.float32))
    return g.reshape(g.shape[0], -1)


def _diag_extract(o, S):
    """o[(4j+n), (S*j+x)] diag blocks -> (N, S) summed over j."""
    r = np.zeros((N, S), np.float32)
    for j in range(4):
        r += o[4 * j:4 * j + 4, S * j:S * (j + 1)]
    return r


def _extract_u(o, S):
    """U part of device output -> (N, S) in hpos order."""
    F = 4 * S
    if U_4D:
        ou = o[:, F:2 * F].reshape(4, N, S, 4)   # [j, n, hpos, j']
        return np.einsum('jnhj->nh', ou)
    return o[:4, F:F + S].copy()


def _finish_core(o, S, wsel, hsel_perm, dsel, mw, mh, md):
    """Host finisher: o is (16, 12S) device output for one (b,c)."""
    F = 4 * S
    T_full = np.zeros((N, DM), np.float32)
    T_full[:, dsel] = _diag_extract(o[:, 0:F], S)
    U_full = np.zeros((N, DM), np.float32)
    U_full[:, hsel_perm] = _extract_u(o, S)
    V_full = np.zeros((N, DM), np.float32)
    V_full[:, wsel] = _diag_extract(o[:, 2 * F:3 * F], S)

    sl_d = T_full * md.astype(np.float32)
    sl_h = U_full * mh.astype(np.float32)
    sl_w = V_full * mw.astype(np.float32)

    def axis_err(sl, mk):
        seg_vals = sl.reshape(N, N_SEG, SEG_W).sum(axis=2, dtype=np.float32)
        seg_cnt = mk.reshape(N, N_SEG, SEG_W).sum(axis=2)
        valid = seg_cnt > 0
        mean = seg_vals / np.where(valid, seg_cnt, 1).astype(np.float32)
        err = np.where(valid, np.maximum(np.float32(1.0) - mean, np.float32(0.0)),
                       np.float32(0.0))
        return err.sum(axis=1, dtype=np.float32)

    e_d = axis_err(sl_d, md)
    e_h = axis_err(sl_h, mh)
    e_w = axis_err(sl_w, mw)
    error = (e_d + e_h + e_w) * np.float32(SEG_W)
    error = np.where(error >= 0, np.square(error), np.float32(0.0))
    return error.sum(dtype=np.float32)


def _prep_core(L, S, wsel, hsel_perm, dsel, mw, mh, md):
    """Per-(b,c) device inputs from the S^3 crop."""
    import ml_dtypes
    f8 = ml_dtypes.float8_e4m3
    Lc = L[np.ix_(wsel, hsel_perm, dsel)]
    lg8 = np.ascontiguousarray(Lc).reshape(S, S * S).astype(f8)
    Lt = L.transpose(2, 1, 0)[np.ix_(dsel, hsel_perm, wsel)]
    lt8 = np.ascontiguousarray(Lt).reshape(S, S * S).astype(f8)

    HP = _hperm(S)
    NQ = S // 8
    mws = mw[:, wsel]          # (N, S)
    mhs = mh[:, hsel_perm]     # already permuted order: b-sel by position
    mds = md[:, dsel]
    b_h = mhs.reshape(N, NQ, 2, 4)          # position p=(q,kt,j) holds mh
    b_w = mws[:, HP].reshape(N, NQ, 2, 4)   # w = wsel[8q+2j+kt]
    gt = _build_g(mws, b_h)
    if U_4D:
        gu = _build_g(mds, b_w)
    else:
        # gu[d, s, kt, n] = md * mw[wsel[2s+kt]] for the 3D pair variant
        gu = np.einsum('np,nsk->pskn', mds.astype(np.float32),
                       mws.reshape(N, S // 2, 2).astype(np.float32)
                       ).reshape(S, -1)
        gu = np.pad(gu, ((0, 0), (0, 4 * S - gu.shape[1])))
    gv = _build_g(mds, b_h)
    g3 = np.concatenate([gt, gu, gv], axis=1)
    ain = np.concatenate(
        [g3.astype(f8), lt8, lg8], axis=1)
    return {"ain": ain}


def kernel(logits: np.ndarray, box_masks: np.ndarray) -> np.ndarray:
    from concourse.bass_utils import run_bass_kernel_spmd

    logits = np.ascontiguousarray(logits, dtype=np.float32)
    mw, mh, md = _host_marginals(box_masks)

    # per-core index selections; fall back to S=128 if any union overflows
    S = S_CROP
    sels = []
    for core in range(N_CORES):
        b, c = divmod(core, C)
        ws = _sel_pad(mw[b, c], S)
        hs = _sel_pad(mh[b, c], S)
        ds = _sel_pad(md[b, c], S)
        if ws is None or hs is None or ds is None:
            sels = None
            break
        sels.append((ws, hs, ds))
    if sels is None:
        S = DM
        ident = np.arange(DM)
        sels = [(ident, ident, ident)] * N_CORES

    if S not in _compiled:
        _compiled[S] = _build(S)
    nc = _compiled[S]

    HP = _hperm(S)
    in_maps, fin = [], []
    for core in range(N_CORES):
        b, c = divmod(core, C)
        ws, hs, ds = sels[core]
        hsp = hs[HP]
        in_maps.append(_prep_core(logits[b, c], S, ws, hsp, ds,
                                  mw[b, c], mh[b, c], md[b, c]))
        fin.append((ws, hsp, ds))

    trace = bool(int(os.environ.get("BOXLOSS_TRACE", "0")))
    res = run_bass_kernel_spmd(nc, in_maps, core_ids=list(range(N_CORES)), trace=trace)
    if trace:
        kernel._last_result = res
        kernel._last_S = S
        kernel._last_sels = fin

    total = np.float32(0.0)
    for core in range(N_CORES):
        b, c = divmod(core, C)
        ws, hsp, ds = fin[core]
        total += _finish_core(np.asarray(res.results[core]["o_all"], np.float32),
                              S, ws, hsp, ds, mw[b, c], mh[b, c], md[b, c])
    return np.float32(total)


# revision 37
# speedup vs baseline: 1.0461x; 1.0461x over previous
"""BoxTightnessPriorLoss Trainium2 kernel (v3: cropped separable DoubleRow).

Inputs (full, host-side):
  logits:    (2, 4, 128, 128, 128) float32   -- (B, C, W, H, D)
  box_masks: (2, 4, 4, 128, 128, 128) bool   -- (B, C, N, W, H, D)

Math: every box mask is a product of three interval indicators
mask[n,w,h,d] = mw[n,w]*mh[n,h]*md[n,d], so each slice profile is a
bilinear contraction of logits with two marginals:
  T_d[n,d] = sum_{w,h} mw mh L      (then sl_d = md * T_d)
  U[n,h]   = sum_{w,d} mw md L      (then sl_h = mh * U)
  V[n,w]   = sum_{h,d} mh md L      (then sl_w = mw * V)
Marginals are exact via stride-16 subsampling (box sides >= 16).

Only logits rows/planes inside the per-axis box-marginal UNION contribute
(all G entries outside are zero), so each core gathers its union indices
(padded with complement indices, whose marginals are zero, to the fixed
crop size S) and the device streams two fp8 layouts of the S^3 crop:
  lg[w', hpos*S + d']   and   lt[d', hpos*S + w']
with h-planes permuted so each 8S-col block q holds planes h = 8q+2j+kt
at position (kt, j), making the T/V passes clean 3D [part, kt, f]
DoubleRow fp8 patterns and U a 4D variant with (h, j) minor order.
Host extracts j-diagonal blocks, scatters through the index selections,
and finishes the tiny per-(b,c,n) segment math in float32.
"""
import os
import numpy as np

B, C, N, DM = 2, 4, 4, 128
SEG_W = 8
N_SEG = DM // SEG_W  # 16
N_CORES = 8

S_CROP = int(os.environ.get("BOXLOSS_S", "104"))  # crop size: multiple
# of 8, >= max per-axis box-union size across cores (seed-0 max is 101;
# kernel() falls back to the full S=128 program if any union overflows).

# U-pass moving AP is 4D; set 0 for the 3D no-j-block fallback (sim-able).
U_4D = bool(int(os.environ.get("BOXLOSS_U4D", "1")))

_compiled = {}  # S -> nc


def _hperm(S):
    p = np.arange(S)
    return (8 * (p >> 3) + 2 * (p & 3) + ((p >> 2) & 1)).astype(np.int64)


def _install_wait_split_patch():
    """This container's walrus (CoreV3) allows only ONE sync-wait per
    instruction; TileContext can attach several.  Split any instruction
    carrying N>1 waits into N-1 preceding wait-only NoOps (same engine)."""
    import concourse.tile as _tile
    import concourse.mybir as _mybir

    if getattr(_tile.TileContext, "_ant_wait_split", False):
        return
    _orig = _tile.TileContext.schedule_and_allocate

    def _split_multi_waits(nc):
        for func in nc.m.functions:
            for bb in func.blocks:
                insts = bb.instructions
                i = 0
                while i < len(insts):
                    inst = insts[i]
                    si = getattr(inst, "sync_info", None)
                    if si is not None and si.on_wait and len(si.on_wait) > 1:
                        waits = list(si.on_wait)
                        si.on_wait = [waits[-1]]
                        nops = []
                        for w in waits[:-1]:
                            nop = _mybir.InstNoOp(
                                name=nc.get_next_instruction_name(),
                                engine=inst.engine,
                                sync_info=_mybir.SyncInfo(on_wait=[w], on_update=[]),
                                bass_nofuse=True,
                            )
                            nops.append(nop)
                            nc.register_instruction(nop, overwrite=True)
                        insts[i:i] = nops
                        i += len(nops)
                    i += 1

    def _patched(self, *a, **kw):
        ret = _orig(self, *a, **kw)
        _split_multi_waits(self.nc)
        return ret

    _tile.TileContext.schedule_and_allocate = _patched
    _tile.TileContext._ant_wait_split = True


def _build(S):
    import concourse.bass as bass
    import concourse.tile as tile
    from concourse import mybir

    _install_wait_split_patch()

    f32 = mybir.dt.float32
    bf16 = mybir.dt.bfloat16
    f8 = mybir.dt.float8e4
    wide = mybir.dt.uint64  # DMA APs bitcast to wide elements
    DR = mybir.MatmulPerfMode.DoubleRow

    NQ = S // 8         # q-blocks per pass
    BL = 8 * S          # cols per q-block
    F = 4 * S           # out free size per pass
    COLS = S * S

    nc = bass.Bass()
    # single concatenated input: [G (3F cols) | lt (COLS) | lg (COLS)]
    GOFF, LTOFF, LGOFF = 0, 3 * F, 3 * F + COLS
    TOT = 3 * F + 2 * COLS
    ain = nc.dram_tensor("ain", [S, TOT], f8, kind="ExternalInput")
    o_all = nc.dram_tensor("o_all", [16, 3 * F], f32, kind="ExternalOutput")

    # stream chunks ~0.5 MiB: readiness latency scales with chunk size,
    # while per-dma_start issue cost is ~0.8us -- this balances both.
    edges = [0, LTOFF + 5 * BL, LTOFF + 10 * BL, LGOFF,
             LGOFF + 5 * BL, LGOFF + 10 * BL, LGOFF + (NQ - 1) * BL, TOT]

    with tile.TileContext(nc) as tc:
        with (
            tc.tile_pool(name="consts", bufs=1) as consts,
            tc.tile_pool(name="big", bufs=1) as big,
            tc.tile_pool(name="outs", bufs=1) as outs,
            tc.tile_pool(name="wpsum", bufs=2, space="PSUM") as wpsum,
            tc.tile_pool(name="apsum", bufs=1, space="PSUM") as apsum,
        ):
            # warm-up source tile; contents are irrelevant (never read back)
            warm = consts.tile([DM, 512], bf16)
            nc.vector.memset(warm[:], 0.0)

            ALL = big.tile([S, TOT], f8)

            # ---- input stream, one HWDGE ring (sync), FIFO chunks
            for lo, hi in zip(edges[:-1], edges[1:]):
                nc.sync.dma_start(out=ALL[:, lo:hi].bitcast(wide),
                                  in_=ain[:, lo:hi].bitcast(wide))

            # ---- PE clock warm-up while DMAs stream
            for i in range(12):
                wp = wpsum.tile([1, 512], f32, tag="wp")
                nc.tensor.matmul(wp[:], warm[:, :1], warm[:], start=True, stop=True)

            p_t = apsum.tile([16, F], f32)
            p_u = apsum.tile([16, F], f32)
            p_v = apsum.tile([16, F], f32)

            def dr3(pout, gcol, loff, q, start, stop):
                nc.tensor.matmul(
                    pout,
                    ALL[:, gcol + 32 * q:gcol + 32 * (q + 1)].rearrange(
                        "p (kt m) -> p kt m", kt=2),
                    ALL[:, loff + BL * q:loff + BL * (q + 1)].rearrange(
                        "p (kt f) -> p kt f", kt=2),
                    start=start, stop=stop, perf_mode=DR,
                )

            # ---- V pass (lt, pipelined on its chunks)
            for q in range(NQ):
                dr3(p_v[:], 2 * F, LTOFF, q, q == 0, q == NQ - 1)

            # ---- U pass (lt, needs full lt); f-order (h, j) keeps the
            # moving feed in 8-byte contiguous runs.
            if U_4D:
                Lt_u = ALL[:, LTOFF:LTOFF + COLS].rearrange(
                    "p (h qq j kt) -> p qq kt h j", qq=NQ, j=4, kt=2)
                for q in range(NQ):
                    nc.tensor.matmul(
                        p_u[:],
                        ALL[:, F + 32 * q:F + 32 * (q + 1)].rearrange(
                            "p (kt m) -> p kt m", kt=2),
                        Lt_u[:, q],
                        start=(q == 0), stop=(q == NQ - 1), perf_mode=DR,
                    )
            else:
                Lt_u3 = ALL[:, LTOFF:LTOFF + COLS].rearrange(
                    "p (h s kt) -> p s kt h", s=S // 2, kt=2)
                for s in range(S // 2):
                    nc.tensor.matmul(
                        p_u[:4, :S],
                        ALL[:, F + 8 * s:F + 8 * (s + 1)].rearrange(
                            "p (kt m) -> p kt m", kt=2),
                        Lt_u3[:, s],
                        start=(s == 0), stop=(s == S // 2 - 1), perf_mode=DR,
                    )

            # ---- T pass (lg, pipelined on its chunks)
            for q in range(NQ):
                dr3(p_t[:], 0, LGOFF, q, q == 0, q == NQ - 1)

            stage = outs.tile([16, 3 * F], f32)
            nc.vector.tensor_copy(stage[:, 2 * F:3 * F], p_v[:])
            if U_4D:
                nc.vector.tensor_copy(stage[:, F:2 * F], p_u[:])
            else:
                nc.vector.memset(stage[:, F:2 * F], 0.0)
                nc.vector.tensor_copy(stage[:4, F:F + S], p_u[:4, :S])
            # ship U+V as soon as staged; T separately at the end
            nc.sync.dma_start(out=o_all[:, F:3 * F], in_=stage[:, F:3 * F])
            nc.vector.tensor_copy(stage[:, 0:F], p_t[:])
            nc.sync.dma_start(out=o_all[:, 0:F], in_=stage[:, 0:F])

    return nc


def _host_marginals(box_masks):
    """Exact interval marginals via stride-16 subsampling (sides >= 16)."""
    mw = box_masks[:, :, :, :, ::16, ::16].any(axis=(4, 5))  # (B,C,N,W)
    mh = box_masks[:, :, :, ::16, :, ::16].any(axis=(3, 5))  # (B,C,N,H)
    md = box_masks[:, :, :, ::16, ::16, :].any(axis=(3, 4))  # (B,C,N,D)
    return mw, mh, md


def _sel_pad(m, S):
    """Union indices of (N, 128) marginal, padded to S with complement."""
    u = m.any(axis=0)
    idx = np.nonzero(u)[0]
    if len(idx) > S:
        return None
    comp = np.nonzero(~u)[0]
    return np.concatenate([idx, comp[:S - len(idx)]])


def _build_g(a, b_sel):
    """G[part, q, kt, j, n] = a[n, part] * b_sel[n, q, kt, j] -> (S, 4S)."""
    g = np.einsum('np,nqkj->pqkjn', a.astype(np.float32),
                  b_sel.astype(np.float32))
    return g.reshape(g.shape[0], -1)


def _diag_extract(o, S):
    """o[(4j+n), (S*j+x)] diag blocks -> (N, S) summed over j."""
    r = np.zeros((N, S), np.float32)
    for j in range(4):
        r += o[4 * j:4 * j + 4, S * j:S * (j + 1)]
    return r


def _extract_u(o, S):
    """U part of device output -> (N, S) in hpos order."""
    F = 4 * S
    if U_4D:
        ou = o[:, F:2 * F].reshape(4, N, S, 4)   # [j, n, hpos, j']
        return np.einsum('jnhj->nh', ou)
    return o[:4, F:F + S].copy()


def _finish_core(o, S, wsel, hsel_perm, dsel, mw, mh, md):
    """Host finisher: o is (16, 12S) device output for one (b,c)."""
    F = 4 * S
    T_full = np.zeros((N, DM), np.float32)
    T_full[:, dsel] = _diag_extract(o[:, 0:F], S)
    U_full = np.zeros((N, DM), np.float32)
    U_full[:, hsel_perm] = _extract_u(o, S)
    V_full = np.zeros((N, DM), np.float32)
    V_full[:, wsel] = _diag_extract(o[:, 2 * F:3 * F], S)

    sl_d = T_full * md.astype(np.float32)
    sl_h = U_full * mh.astype(np.float32)
    sl_w = V_full * mw.astype(np.float32)

    def axis_err(sl, mk):
        seg_vals = sl.reshape(N, N_SEG, SEG_W).sum(axis=2, dtype=np.float32)
        seg_cnt = mk.reshape(N, N_SEG, SEG_W).sum(axis=2)
        valid = seg_cnt > 0
        mean = seg_vals / np.where(valid, seg_cnt, 1).astype(np.float32)
        err = np.where(valid, np.maximum(np.float32(1.0) - mean, np.float32(0.0)),
                       np.float32(0.0))
        return err.sum(axis=1, dtype=np.float32)

    e_d = axis_err(sl_d, md)
    e_h = axis_err(sl_h, mh)
    e_w = axis_err(sl_w, mw)
    error = (e_d + e_h + e_w) * np.float32(SEG_W)
    error = np.where(error >= 0, np.square(error), np.float32(0.0))
    return error.sum(dtype=np.float32)


def _prep_core(L, S, wsel, hsel_perm, dsel, mw, mh, md):
    """Per-(b,c) device inputs from the S^3 crop."""
    import ml_dtypes
    f8 = ml_dtypes.float8_e4m3
    Lc = L[np.ix_(wsel, hsel_perm, dsel)]
    lg8 = np.ascontiguousarray(Lc).reshape(S, S * S).astype(f8)
    Lt = L.transpose(2, 1, 0)[np.ix_(dsel, hsel_perm, wsel)]
    lt8 = np.ascontiguousarray(Lt).reshape(S, S * S).astype(f8)

    HP = _hperm(S)
    NQ = S // 8
    mws = mw[:, wsel]          # (N, S)
    mhs = mh[:, hsel_perm]     # already permuted order: b-sel by position
    mds = md[:, dsel]
    b_h = mhs.reshape(N, NQ, 2, 4)          # position p=(q,kt,j) holds mh
    b_w = mws[:, HP].reshape(N, NQ, 2, 4)   # w = wsel[8q+2j+kt]
    gt = _build_g(mws, b_h)
    if U_4D:
        gu = _build_g(mds, b_w)
    else:
        # gu[d, s, kt, n] = md * mw[wsel[2s+kt]] for the 3D pair variant
        gu = np.einsum('np,nsk->pskn', mds.astype(np.float32),
                       mws.reshape(N, S // 2, 2).astype(np.float32)
                       ).reshape(S, -1)
        gu = np.pad(gu, ((0, 0), (0, 4 * S - gu.shape[1])))
    gv = _build_g(mds, b_h)
    g3 = np.concatenate([gt, gu, gv], axis=1)
    ain = np.concatenate(
        [g3.astype(f8), lt8, lg8], axis=1)
    return {"ain": ain}


def kernel(logits: np.ndarray, box_masks: np.ndarray) -> np.ndarray:
    from concourse.bass_utils import run_bass_kernel_spmd

    logits = np.ascontiguousarray(logits, dtype=np.float32)
    box_masks = np.asarray(box_masks)
    mw, mh, md = _host_marginals(box_masks)

    # per-core index selections; fall back to S=128 if any union overflows
    S = S_CROP
    sels = []
    for core in range(N_CORES):
        b, c = divmod(core, C)
        ws = _sel_pad(mw[b, c], S)
        hs = _sel_pad(mh[b, c], S)
        ds = _sel_pad(md[b, c], S)
        if ws is None or hs is None or ds is None:
            sels = None
            break
        sels.append((ws, hs, ds))
    if sels is None:
        S = DM
        ident = np.arange(DM)
        sels = [(ident, ident, ident)] * N_CORES

    if S not in _compiled:
        _compiled[S] = _build(S)
    nc = _compiled[S]

    HP = _hperm(S)
    in_maps, fin = [], []
    for core in range(N_CORES):
        b, c = divmod(core, C)
        ws, hs, ds = sels[core]
        hsp = hs[HP]
        in_maps.append(_prep_core(logits[b, c], S, ws, hsp, ds,
                                  mw[b, c], mh[b, c], md[b, c]))
        fin.append((ws, hsp, ds))

    trace = bool(int(os.environ.get("BOXLOSS_TRACE", "0")))
    res = run_bass_kernel_spmd(nc, in_maps, core_ids=list(range(N_CORES)), trace=trace)
    if trace:
        kernel._last_result = res
        kernel._last_S = S
        kernel._last_sels = fin

    total = np.float32(0.0)
    for core in range(N_CORES):
        b, c = divmod(core, C)
        ws, hsp, ds = fin[core]
        total += _finish_core(np.asarray(res.results[core]["o_all"], np.float32),
                              S, ws, hsp, ds, mw[b, c], mh[b, c], md[b, c])
    return np.float32(total)


# revision 38
# speedup vs baseline: 1.0671x; 1.0201x over previous
"""BoxTightnessPriorLoss Trainium2 kernel (v3: cropped separable DoubleRow).

Inputs (full, host-side):
  logits:    (2, 4, 128, 128, 128) float32   -- (B, C, W, H, D)
  box_masks: (2, 4, 4, 128, 128, 128) bool   -- (B, C, N, W, H, D)

Math: every box mask is a product of three interval indicators
mask[n,w,h,d] = mw[n,w]*mh[n,h]*md[n,d], so each slice profile is a
bilinear contraction of logits with two marginals:
  T_d[n,d] = sum_{w,h} mw mh L      (then sl_d = md * T_d)
  U[n,h]   = sum_{w,d} mw md L      (then sl_h = mh * U)
  V[n,w]   = sum_{h,d} mh md L      (then sl_w = mw * V)
Marginals are exact via stride-16 subsampling (box sides >= 16).

Only logits rows/planes inside the per-axis box-marginal UNION contribute
(all G entries outside are zero), so each core gathers its union indices
(padded with complement indices, whose marginals are zero, to the fixed
crop size S) and the device streams two fp8 layouts of the S^3 crop:
  lg[w', hpos*S + d']   and   lt[d', hpos*S + w']
with h-planes permuted so each 8S-col block q holds planes h = 8q+2j+kt
at position (kt, j), making the T/V passes clean 3D [part, kt, f]
DoubleRow fp8 patterns and U a 4D variant with (h, j) minor order.
Host extracts j-diagonal blocks, scatters through the index selections,
and finishes the tiny per-(b,c,n) segment math in float32.
"""
import os
import numpy as np

B, C, N, DM = 2, 4, 4, 128
SEG_W = 8
N_SEG = DM // SEG_W  # 16
N_CORES = 8

S_CROP = int(os.environ.get("BOXLOSS_S", "104"))  # crop size: multiple
# of 8, >= max per-axis box-union size across cores (seed-0 max is 101;
# kernel() falls back to the full S=128 program if any union overflows).

# U-pass moving AP is 4D; set 0 for the 3D no-j-block fallback (sim-able).
U_4D = bool(int(os.environ.get("BOXLOSS_U4D", "1")))

_compiled = {}  # S -> nc


def _hperm(S):
    p = np.arange(S)
    return (8 * (p >> 3) + 2 * (p & 3) + ((p >> 2) & 1)).astype(np.int64)


def _install_wait_split_patch():
    """This container's walrus (CoreV3) allows only ONE sync-wait per
    instruction; TileContext can attach several.  Split any instruction
    carrying N>1 waits into N-1 preceding wait-only NoOps (same engine)."""
    import concourse.tile as _tile
    import concourse.mybir as _mybir

    if getattr(_tile.TileContext, "_ant_wait_split", False):
        return
    _orig = _tile.TileContext.schedule_and_allocate

    def _split_multi_waits(nc):
        for func in nc.m.functions:
            for bb in func.blocks:
                insts = bb.instructions
                i = 0
                while i < len(insts):
                    inst = insts[i]
                    si = getattr(inst, "sync_info", None)
                    if si is not None and si.on_wait and len(si.on_wait) > 1:
                        waits = list(si.on_wait)
                        si.on_wait = [waits[-1]]
                        nops = []
                        for w in waits[:-1]:
                            nop = _mybir.InstNoOp(
                                name=nc.get_next_instruction_name(),
                                engine=inst.engine,
                                sync_info=_mybir.SyncInfo(on_wait=[w], on_update=[]),
                                bass_nofuse=True,
                            )
                            nops.append(nop)
                            nc.register_instruction(nop, overwrite=True)
                        insts[i:i] = nops
                        i += len(nops)
                    i += 1

    def _patched(self, *a, **kw):
        ret = _orig(self, *a, **kw)
        _split_multi_waits(self.nc)
        return ret

    _tile.TileContext.schedule_and_allocate = _patched
    _tile.TileContext._ant_wait_split = True


def _build(S):
    import concourse.bass as bass
    import concourse.tile as tile
    from concourse import mybir

    _install_wait_split_patch()

    f32 = mybir.dt.float32
    bf16 = mybir.dt.bfloat16
    f8 = mybir.dt.float8e4
    wide = mybir.dt.uint64  # DMA APs bitcast to wide elements
    DR = mybir.MatmulPerfMode.DoubleRow

    NQ = S // 8         # q-blocks per pass
    BL = 8 * S          # cols per q-block
    F = 4 * S           # out free size per pass
    COLS = S * S

    nc = bass.Bass()
    # single concatenated input: [G (3F cols) | lt (COLS) | lg (COLS)]
    GOFF, LTOFF, LGOFF = 0, 3 * F, 3 * F + COLS
    TOT = 3 * F + 2 * COLS
    ain = nc.dram_tensor("ain", [S, TOT], f8, kind="ExternalInput")
    o_all = nc.dram_tensor("o_all", [16, 3 * F], f32, kind="ExternalOutput")

    # stream chunks ~0.5 MiB: readiness latency scales with chunk size,
    # while per-dma_start issue cost is ~0.8us -- this balances both.
    edges = [0, LTOFF + 5 * BL, LTOFF + 10 * BL, LGOFF,
             LGOFF + 5 * BL, LGOFF + 10 * BL, LGOFF + (NQ - 1) * BL, TOT]

    with tile.TileContext(nc) as tc:
        with (
            tc.tile_pool(name="consts", bufs=1) as consts,
            tc.tile_pool(name="big", bufs=1) as big,
            tc.tile_pool(name="outs", bufs=1) as outs,
            tc.tile_pool(name="wpsum", bufs=2, space="PSUM") as wpsum,
            tc.tile_pool(name="apsum", bufs=1, space="PSUM") as apsum,
        ):
            # warm-up source tile; contents are irrelevant (never read back)
            warm = consts.tile([DM, 512], bf16)
            nc.vector.memset(warm[:], 0.0)

            ALL = big.tile([S, TOT], f8)

            # ---- input stream, one HWDGE ring (sync), FIFO chunks
            for lo, hi in zip(edges[:-1], edges[1:]):
                nc.sync.dma_start(out=ALL[:, lo:hi].bitcast(wide),
                                  in_=ain[:, lo:hi].bitcast(wide))

            # ---- PE clock warm-up while DMAs stream
            for i in range(12):
                wp = wpsum.tile([1, 512], f32, tag="wp")
                nc.tensor.matmul(wp[:], warm[:, :1], warm[:], start=True, stop=True)

            p_t = apsum.tile([16, F], f32)
            p_u = apsum.tile([16, F], f32)
            p_v = apsum.tile([16, F], f32)

            def dr3(pout, gcol, loff, q, start, stop):
                nc.tensor.matmul(
                    pout,
                    ALL[:, gcol + 32 * q:gcol + 32 * (q + 1)].rearrange(
                        "p (kt m) -> p kt m", kt=2),
                    ALL[:, loff + BL * q:loff + BL * (q + 1)].rearrange(
                        "p (kt f) -> p kt f", kt=2),
                    start=start, stop=stop, perf_mode=DR,
                )

            # ---- V pass (lt, pipelined on its chunks)
            for q in range(NQ):
                dr3(p_v[:], 2 * F, LTOFF, q, q == 0, q == NQ - 1)

            # ---- U pass (lt, needs full lt); f-order (h, j) keeps the
            # moving feed in 8-byte contiguous runs.
            if U_4D:
                Lt_u = ALL[:, LTOFF:LTOFF + COLS].rearrange(
                    "p (h qq j kt) -> p qq kt h j", qq=NQ, j=4, kt=2)
                for q in range(NQ):
                    nc.tensor.matmul(
                        p_u[:],
                        ALL[:, F + 32 * q:F + 32 * (q + 1)].rearrange(
                            "p (kt m) -> p kt m", kt=2),
                        Lt_u[:, q],
                        start=(q == 0), stop=(q == NQ - 1), perf_mode=DR,
                    )
            else:
                Lt_u3 = ALL[:, LTOFF:LTOFF + COLS].rearrange(
                    "p (h s kt) -> p s kt h", s=S // 2, kt=2)
                for s in range(S // 2):
                    nc.tensor.matmul(
                        p_u[:4, :S],
                        ALL[:, F + 8 * s:F + 8 * (s + 1)].rearrange(
                            "p (kt m) -> p kt m", kt=2),
                        Lt_u3[:, s],
                        start=(s == 0), stop=(s == S // 2 - 1), perf_mode=DR,
                    )

            # ---- T pass (lg, pipelined on its chunks)
            for q in range(NQ):
                dr3(p_t[:], 0, LGOFF, q, q == 0, q == NQ - 1)

            stage = outs.tile([16, 3 * F], f32)
            nc.vector.tensor_copy(stage[:, 2 * F:3 * F], p_v[:])
            if U_4D:
                nc.vector.tensor_copy(stage[:, F:2 * F], p_u[:])
            else:
                nc.vector.memset(stage[:, F:2 * F], 0.0)
                nc.vector.tensor_copy(stage[:4, F:F + S], p_u[:4, :S])
            # ship U+V on the Act ring so the sync ring is idle when the
            # critical final o_t DMA needs it
            nc.scalar.dma_start(out=o_all[:, F:3 * F], in_=stage[:, F:3 * F])
            nc.vector.tensor_copy(stage[:, 0:F], p_t[:])
            nc.sync.dma_start(out=o_all[:, 0:F], in_=stage[:, 0:F])

    return nc


def _host_marginals(box_masks):
    """Exact interval marginals via stride-16 subsampling (sides >= 16)."""
    mw = box_masks[:, :, :, :, ::16, ::16].any(axis=(4, 5))  # (B,C,N,W)
    mh = box_masks[:, :, :, ::16, :, ::16].any(axis=(3, 5))  # (B,C,N,H)
    md = box_masks[:, :, :, ::16, ::16, :].any(axis=(3, 4))  # (B,C,N,D)
    return mw, mh, md


def _sel_pad(m, S):
    """Union indices of (N, 128) marginal, padded to S with complement."""
    u = m.any(axis=0)
    idx = np.nonzero(u)[0]
    if len(idx) > S:
        return None
    comp = np.nonzero(~u)[0]
    return np.concatenate([idx, comp[:S - len(idx)]])


def _build_g(a, b_sel):
    """G[part, q, kt, j, n] = a[n, part] * b_sel[n, q, kt, j] -> (S, 4S)."""
    g = np.einsum('np,nqkj->pqkjn', a.astype(np.float32),
                  b_sel.astype(np.float32))
    return g.reshape(g.shape[0], -1)


def _diag_extract(o, S):
    """o[(4j+n), (S*j+x)] diag blocks -> (N, S) summed over j."""
    r = np.zeros((N, S), np.float32)
    for j in range(4):
        r += o[4 * j:4 * j + 4, S * j:S * (j + 1)]
    return r


def _extract_u(o, S):
    """U part of device output -> (N, S) in hpos order."""
    F = 4 * S
    if U_4D:
        ou = o[:, F:2 * F].reshape(4, N, S, 4)   # [j, n, hpos, j']
        return np.einsum('jnhj->nh', ou)
    return o[:4, F:F + S].copy()


def _finish_core(o, S, wsel, hsel_perm, dsel, mw, mh, md):
    """Host finisher: o is (16, 12S) device output for one (b,c)."""
    F = 4 * S
    T_full = np.zeros((N, DM), np.float32)
    T_full[:, dsel] = _diag_extract(o[:, 0:F], S)
    U_full = np.zeros((N, DM), np.float32)
    U_full[:, hsel_perm] = _extract_u(o, S)
    V_full = np.zeros((N, DM), np.float32)
    V_full[:, wsel] = _diag_extract(o[:, 2 * F:3 * F], S)

    sl_d = T_full * md.astype(np.float32)
    sl_h = U_full * mh.astype(np.float32)
    sl_w = V_full * mw.astype(np.float32)

    def axis_err(sl, mk):
        seg_vals = sl.reshape(N, N_SEG, SEG_W).sum(axis=2, dtype=np.float32)
        seg_cnt = mk.reshape(N, N_SEG, SEG_W).sum(axis=2)
        valid = seg_cnt > 0
        mean = seg_vals / np.where(valid, seg_cnt, 1).astype(np.float32)
        err = np.where(valid, np.maximum(np.float32(1.0) - mean, np.float32(0.0)),
                       np.float32(0.0))
        return err.sum(axis=1, dtype=np.float32)

    e_d = axis_err(sl_d, md)
    e_h = axis_err(sl_h, mh)
    e_w = axis_err(sl_w, mw)
    error = (e_d + e_h + e_w) * np.float32(SEG_W)
    error = np.where(error >= 0, np.square(error), np.float32(0.0))
    return error.sum(dtype=np.float32)


def _prep_core(L, S, wsel, hsel_perm, dsel, mw, mh, md):
    """Per-(b,c) device inputs from the S^3 crop."""
    import ml_dtypes
    f8 = ml_dtypes.float8_e4m3
    Lc = L[np.ix_(wsel, hsel_perm, dsel)]
    lg8 = np.ascontiguousarray(Lc).reshape(S, S * S).astype(f8)
    Lt = L.transpose(2, 1, 0)[np.ix_(dsel, hsel_perm, wsel)]
    lt8 = np.ascontiguousarray(Lt).reshape(S, S * S).astype(f8)

    HP = _hperm(S)
    NQ = S // 8
    mws = mw[:, wsel]          # (N, S)
    mhs = mh[:, hsel_perm]     # already permuted order: b-sel by position
    mds = md[:, dsel]
    b_h = mhs.reshape(N, NQ, 2, 4)          # position p=(q,kt,j) holds mh
    b_w = mws[:, HP].reshape(N, NQ, 2, 4)   # w = wsel[8q+2j+kt]
    gt = _build_g(mws, b_h)
    if U_4D:
        gu = _build_g(mds, b_w)
    else:
        # gu[d, s, kt, n] = md * mw[wsel[2s+kt]] for the 3D pair variant
        gu = np.einsum('np,nsk->pskn', mds.astype(np.float32),
                       mws.reshape(N, S // 2, 2).astype(np.float32)
                       ).reshape(S, -1)
        gu = np.pad(gu, ((0, 0), (0, 4 * S - gu.shape[1])))
    gv = _build_g(mds, b_h)
    g3 = np.concatenate([gt, gu, gv], axis=1)
    ain = np.concatenate(
        [g3.astype(f8), lt8, lg8], axis=1)
    return {"ain": ain}


def kernel(logits: np.ndarray, box_masks: np.ndarray) -> np.ndarray:
    from concourse.bass_utils import run_bass_kernel_spmd

    logits = np.ascontiguousarray(logits, dtype=np.float32)
    box_masks = np.asarray(box_masks)
    mw, mh, md = _host_marginals(box_masks)

    # per-core index selections; fall back to S=128 if any union overflows
    S = S_CROP
    sels = []
    for core in range(N_CORES):
        b, c = divmod(core, C)
        ws = _sel_pad(mw[b, c], S)
        hs = _sel_pad(mh[b, c], S)
        ds = _sel_pad(md[b, c], S)
        if ws is None or hs is None or ds is None:
            sels = None
            break
        sels.append((ws, hs, ds))
    if sels is None:
        S = DM
        ident = np.arange(DM)
        sels = [(ident, ident, ident)] * N_CORES

    if S not in _compiled:
        _compiled[S] = _build(S)
    nc = _compiled[S]

    HP = _hperm(S)
    in_maps, fin = [], []
    for core in range(N_CORES):
        b, c = divmod(core, C)
        ws, hs, ds = sels[core]
        hsp = hs[HP]
        in_maps.append(_prep_core(logits[b, c], S, ws, hsp, ds,
                                  mw[b, c], mh[b, c], md[b, c]))
        fin.append((ws, hsp, ds))

    trace = bool(int(os.environ.get("BOXLOSS_TRACE", "0")))
    res = run_bass_kernel_spmd(nc, in_maps, core_ids=list(range(N_CORES)), trace=trace)
    if trace:
        kernel._last_result = res
        kernel._last_S = S
        kernel._last_sels = fin

    total = np.float32(0.0)
    for core in range(N_CORES):
        b, c = divmod(core, C)
        ws, hsp, ds = fin[core]
        total += _finish_core(np.asarray(res.results[core]["o_all"], np.float32),
                              S, ws, hsp, ds, mw[b, c], mh[b, c], md[b, c])
    return np.float32(total)


# revision 39
# speedup vs baseline: 1.0800x; 1.0121x over previous
"""BoxTightnessPriorLoss Trainium2 kernel (v3: cropped separable DoubleRow).

Inputs (full, host-side):
  logits:    (2, 4, 128, 128, 128) float32   -- (B, C, W, H, D)
  box_masks: (2, 4, 4, 128, 128, 128) bool   -- (B, C, N, W, H, D)

Math: every box mask is a product of three interval indicators
mask[n,w,h,d] = mw[n,w]*mh[n,h]*md[n,d], so each slice profile is a
bilinear contraction of logits with two marginals:
  T_d[n,d] = sum_{w,h} mw mh L      (then sl_d = md * T_d)
  U[n,h]   = sum_{w,d} mw md L      (then sl_h = mh * U)
  V[n,w]   = sum_{h,d} mh md L      (then sl_w = mw * V)
Marginals are exact via stride-16 subsampling (box sides >= 16).

Only logits rows/planes inside the per-axis box-marginal UNION contribute
(all G entries outside are zero), so each core gathers its union indices
(padded with complement indices, whose marginals are zero, to the fixed
crop size S) and the device streams two fp8 layouts of the S^3 crop:
  lg[w', hpos*S + d']   and   lt[d', hpos*S + w']
with h-planes permuted so each 8S-col block q holds planes h = 8q+2j+kt
at position (kt, j), making the T/V passes clean 3D [part, kt, f]
DoubleRow fp8 patterns and U a 4D variant with (h, j) minor order.
Host extracts j-diagonal blocks, scatters through the index selections,
and finishes the tiny per-(b,c,n) segment math in float32.
"""
import os
import numpy as np

B, C, N, DM = 2, 4, 4, 128
SEG_W = 8
N_SEG = DM // SEG_W  # 16
N_CORES = 8

S_CROP = int(os.environ.get("BOXLOSS_S", "104"))  # crop size: multiple
# of 8, >= max per-axis box-union size across cores (seed-0 max is 101;
# kernel() falls back to the full S=128 program if any union overflows).

# U-pass moving AP is 4D; set 0 for the 3D no-j-block fallback (sim-able).
U_4D = bool(int(os.environ.get("BOXLOSS_U4D", "1")))

_compiled = {}  # S -> nc


def _hperm(S):
    p = np.arange(S)
    return (8 * (p >> 3) + 2 * (p & 3) + ((p >> 2) & 1)).astype(np.int64)


def _install_wait_split_patch():
    """This container's walrus (CoreV3) allows only ONE sync-wait per
    instruction; TileContext can attach several.  Split any instruction
    carrying N>1 waits into N-1 preceding wait-only NoOps (same engine)."""
    import concourse.tile as _tile
    import concourse.mybir as _mybir

    if getattr(_tile.TileContext, "_ant_wait_split", False):
        return
    _orig = _tile.TileContext.schedule_and_allocate

    def _split_multi_waits(nc):
        for func in nc.m.functions:
            for bb in func.blocks:
                insts = bb.instructions
                i = 0
                while i < len(insts):
                    inst = insts[i]
                    si = getattr(inst, "sync_info", None)
                    if si is not None and si.on_wait and len(si.on_wait) > 1:
                        waits = list(si.on_wait)
                        si.on_wait = [waits[-1]]
                        nops = []
                        for w in waits[:-1]:
                            nop = _mybir.InstNoOp(
                                name=nc.get_next_instruction_name(),
                                engine=inst.engine,
                                sync_info=_mybir.SyncInfo(on_wait=[w], on_update=[]),
                                bass_nofuse=True,
                            )
                            nops.append(nop)
                            nc.register_instruction(nop, overwrite=True)
                        insts[i:i] = nops
                        i += len(nops)
                    i += 1

    def _patched(self, *a, **kw):
        ret = _orig(self, *a, **kw)
        _split_multi_waits(self.nc)
        return ret

    _tile.TileContext.schedule_and_allocate = _patched
    _tile.TileContext._ant_wait_split = True


def _build(S):
    import concourse.bass as bass
    import concourse.tile as tile
    from concourse import mybir

    _install_wait_split_patch()

    f32 = mybir.dt.float32
    bf16 = mybir.dt.bfloat16
    f8 = mybir.dt.float8e4
    wide = mybir.dt.uint64  # DMA APs bitcast to wide elements
    DR = mybir.MatmulPerfMode.DoubleRow

    NQ = S // 8         # q-blocks per pass
    BL = 8 * S          # cols per q-block
    F = 4 * S           # out free size per pass
    COLS = S * S

    nc = bass.Bass()
    # single concatenated input: [G (3F cols) | lt (COLS) | lg (COLS)]
    GOFF, LTOFF, LGOFF = 0, 3 * F, 3 * F + COLS
    TOT = 3 * F + 2 * COLS
    ain = nc.dram_tensor("ain", [S, TOT], f8, kind="ExternalInput")
    o_all = nc.dram_tensor("o_all", [16, 3 * F], f32, kind="ExternalOutput")

    # stream chunks ~0.5 MiB: readiness latency scales with chunk size,
    # while per-dma_start issue cost is ~0.8us -- this balances both.
    edges = [0, LTOFF + 5 * BL, LTOFF + 10 * BL, LGOFF,
             LGOFF + 5 * BL, LGOFF + 10 * BL, LGOFF + (NQ - 1) * BL, TOT]

    with tile.TileContext(nc) as tc:
        with (
            tc.tile_pool(name="consts", bufs=1) as consts,
            tc.tile_pool(name="big", bufs=1) as big,
            tc.tile_pool(name="outs", bufs=1) as outs,
            tc.tile_pool(name="wpsum", bufs=2, space="PSUM") as wpsum,
            tc.tile_pool(name="apsum", bufs=1, space="PSUM") as apsum,
        ):
            # warm-up source tile; contents are irrelevant (never read back)
            warm = consts.tile([DM, 512], bf16)
            nc.vector.memset(warm[:], 0.0)

            ALL = big.tile([S, TOT], f8)

            # ---- input stream, one HWDGE ring (sync), FIFO chunks
            for lo, hi in zip(edges[:-1], edges[1:]):
                nc.sync.dma_start(out=ALL[:, lo:hi].bitcast(wide),
                                  in_=ain[:, lo:hi].bitcast(wide))

            # ---- PE clock warm-up while DMAs stream
            for i in range(12):
                wp = wpsum.tile([1, 512], f32, tag="wp")
                nc.tensor.matmul(wp[:], warm[:, :1], warm[:], start=True, stop=True)

            p_t = apsum.tile([16, F], f32)
            p_u = apsum.tile([16, F], f32)
            p_v = apsum.tile([16, F], f32)

            def dr3(pout, gcol, loff, q, start, stop):
                nc.tensor.matmul(
                    pout,
                    ALL[:, gcol + 32 * q:gcol + 32 * (q + 1)].rearrange(
                        "p (kt m) -> p kt m", kt=2),
                    ALL[:, loff + BL * q:loff + BL * (q + 1)].rearrange(
                        "p (kt f) -> p kt f", kt=2),
                    start=start, stop=stop, perf_mode=DR,
                )

            # ---- V pass (lt, pipelined on its chunks)
            for q in range(NQ):
                dr3(p_v[:], 2 * F, LTOFF, q, q == 0, q == NQ - 1)

            # ---- U pass (lt, needs full lt); f-order (h, j) keeps the
            # moving feed in 8-byte contiguous runs.
            if U_4D:
                Lt_u = ALL[:, LTOFF:LTOFF + COLS].rearrange(
                    "p (h qq j kt) -> p qq kt h j", qq=NQ, j=4, kt=2)
                for q in range(NQ):
                    nc.tensor.matmul(
                        p_u[:],
                        ALL[:, F + 32 * q:F + 32 * (q + 1)].rearrange(
                            "p (kt m) -> p kt m", kt=2),
                        Lt_u[:, q],
                        start=(q == 0), stop=(q == NQ - 1), perf_mode=DR,
                    )
            else:
                Lt_u3 = ALL[:, LTOFF:LTOFF + COLS].rearrange(
                    "p (h s kt) -> p s kt h", s=S // 2, kt=2)
                for s in range(S // 2):
                    nc.tensor.matmul(
                        p_u[:4, :S],
                        ALL[:, F + 8 * s:F + 8 * (s + 1)].rearrange(
                            "p (kt m) -> p kt m", kt=2),
                        Lt_u3[:, s],
                        start=(s == 0), stop=(s == S // 2 - 1), perf_mode=DR,
                    )

            # ---- T pass (lg, pipelined on its chunks)
            for q in range(NQ):
                dr3(p_t[:], 0, LGOFF, q, q == 0, q == NQ - 1)

            stage = outs.tile([16, 3 * F], f32)
            nc.vector.tensor_copy(stage[:, 2 * F:3 * F], p_v[:])
            if U_4D:
                nc.vector.tensor_copy(stage[:, F:2 * F], p_u[:])
            else:
                nc.vector.memset(stage[:, F:2 * F], 0.0)
                nc.vector.tensor_copy(stage[:4, F:F + S], p_u[:4, :S])
            # ship U+V on the Act ring so the sync ring is idle when the
            # critical final o_t DMA needs it
            nc.scalar.dma_start(out=o_all[:, F:3 * F], in_=stage[:, F:3 * F],
                                single_packet=True)
            nc.vector.tensor_copy(stage[:, 0:F], p_t[:])
            nc.sync.dma_start(out=o_all[:, 0:F], in_=stage[:, 0:F],
                              single_packet=True)

    return nc


def _host_marginals(box_masks):
    """Exact interval marginals via stride-16 subsampling (sides >= 16)."""
    mw = box_masks[:, :, :, :, ::16, ::16].any(axis=(4, 5))  # (B,C,N,W)
    mh = box_masks[:, :, :, ::16, :, ::16].any(axis=(3, 5))  # (B,C,N,H)
    md = box_masks[:, :, :, ::16, ::16, :].any(axis=(3, 4))  # (B,C,N,D)
    return mw, mh, md


def _sel_pad(m, S):
    """Union indices of (N, 128) marginal, padded to S with complement."""
    u = m.any(axis=0)
    idx = np.nonzero(u)[0]
    if len(idx) > S:
        return None
    comp = np.nonzero(~u)[0]
    return np.concatenate([idx, comp[:S - len(idx)]])


def _build_g(a, b_sel):
    """G[part, q, kt, j, n] = a[n, part] * b_sel[n, q, kt, j] -> (S, 4S)."""
    g = np.einsum('np,nqkj->pqkjn', a.astype(np.float32),
                  b_sel.astype(np.float32))
    return g.reshape(g.shape[0], -1)


def _diag_extract(o, S):
    """o[(4j+n), (S*j+x)] diag blocks -> (N, S) summed over j."""
    r = np.zeros((N, S), np.float32)
    for j in range(4):
        r += o[4 * j:4 * j + 4, S * j:S * (j + 1)]
    return r


def _extract_u(o, S):
    """U part of device output -> (N, S) in hpos order."""
    F = 4 * S
    if U_4D:
        ou = o[:, F:2 * F].reshape(4, N, S, 4)   # [j, n, hpos, j']
        return np.einsum('jnhj->nh', ou)
    return o[:4, F:F + S].copy()


def _finish_core(o, S, wsel, hsel_perm, dsel, mw, mh, md):
    """Host finisher: o is (16, 12S) device output for one (b,c)."""
    F = 4 * S
    T_full = np.zeros((N, DM), np.float32)
    T_full[:, dsel] = _diag_extract(o[:, 0:F], S)
    U_full = np.zeros((N, DM), np.float32)
    U_full[:, hsel_perm] = _extract_u(o, S)
    V_full = np.zeros((N, DM), np.float32)
    V_full[:, wsel] = _diag_extract(o[:, 2 * F:3 * F], S)

    sl_d = T_full * md.astype(np.float32)
    sl_h = U_full * mh.astype(np.float32)
    sl_w = V_full * mw.astype(np.float32)

    def axis_err(sl, mk):
        seg_vals = sl.reshape(N, N_SEG, SEG_W).sum(axis=2, dtype=np.float32)
        seg_cnt = mk.reshape(N, N_SEG, SEG_W).sum(axis=2)
        valid = seg_cnt > 0
        mean = seg_vals / np.where(valid, seg_cnt, 1).astype(np.float32)
        err = np.where(valid, np.maximum(np.float32(1.0) - mean, np.float32(0.0)),
                       np.float32(0.0))
        return err.sum(axis=1, dtype=np.float32)

    e_d = axis_err(sl_d, md)
    e_h = axis_err(sl_h, mh)
    e_w = axis_err(sl_w, mw)
    error = (e_d + e_h + e_w) * np.float32(SEG_W)
    error = np.where(error >= 0, np.square(error), np.float32(0.0))
    return error.sum(dtype=np.float32)


def _prep_core(L, S, wsel, hsel_perm, dsel, mw, mh, md):
    """Per-(b,c) device inputs from the S^3 crop."""
    import ml_dtypes
    f8 = ml_dtypes.float8_e4m3
    Lc = L[np.ix_(wsel, hsel_perm, dsel)]
    lg8 = np.ascontiguousarray(Lc).reshape(S, S * S).astype(f8)
    Lt = L.transpose(2, 1, 0)[np.ix_(dsel, hsel_perm, wsel)]
    lt8 = np.ascontiguousarray(Lt).reshape(S, S * S).astype(f8)

    HP = _hperm(S)
    NQ = S // 8
    mws = mw[:, wsel]          # (N, S)
    mhs = mh[:, hsel_perm]     # already permuted order: b-sel by position
    mds = md[:, dsel]
    b_h = mhs.reshape(N, NQ, 2, 4)          # position p=(q,kt,j) holds mh
    b_w = mws[:, HP].reshape(N, NQ, 2, 4)   # w = wsel[8q+2j+kt]
    gt = _build_g(mws, b_h)
    if U_4D:
        gu = _build_g(mds, b_w)
    else:
        # gu[d, s, kt, n] = md * mw[wsel[2s+kt]] for the 3D pair variant
        gu = np.einsum('np,nsk->pskn', mds.astype(np.float32),
                       mws.reshape(N, S // 2, 2).astype(np.float32)
                       ).reshape(S, -1)
        gu = np.pad(gu, ((0, 0), (0, 4 * S - gu.shape[1])))
    gv = _build_g(mds, b_h)
    g3 = np.concatenate([gt, gu, gv], axis=1)
    ain = np.concatenate(
        [g3.astype(f8), lt8, lg8], axis=1)
    return {"ain": ain}


def kernel(logits: np.ndarray, box_masks: np.ndarray) -> np.ndarray:
    from concourse.bass_utils import run_bass_kernel_spmd

    logits = np.ascontiguousarray(logits, dtype=np.float32)
    box_masks = np.asarray(box_masks)
    mw, mh, md = _host_marginals(box_masks)

    # per-core index selections; fall back to S=128 if any union overflows
    S = S_CROP
    sels = []
    for core in range(N_CORES):
        b, c = divmod(core, C)
        ws = _sel_pad(mw[b, c], S)
        hs = _sel_pad(mh[b, c], S)
        ds = _sel_pad(md[b, c], S)
        if ws is None or hs is None or ds is None:
            sels = None
            break
        sels.append((ws, hs, ds))
    if sels is None:
        S = DM
        ident = np.arange(DM)
        sels = [(ident, ident, ident)] * N_CORES

    if S not in _compiled:
        _compiled[S] = _build(S)
    nc = _compiled[S]

    HP = _hperm(S)
    in_maps, fin = [], []
    for core in range(N_CORES):
        b, c = divmod(core, C)
        ws, hs, ds = sels[core]
        hsp = hs[HP]
        in_maps.append(_prep_core(logits[b, c], S, ws, hsp, ds,
                                  mw[b, c], mh[b, c], md[b, c]))
        fin.append((ws, hsp, ds))

    trace = bool(int(os.environ.get("BOXLOSS_TRACE", "0")))
    res = run_bass_kernel_spmd(nc, in_maps, core_ids=list(range(N_CORES)), trace=trace)
    if trace:
        kernel._last_result = res
        kernel._last_S = S
        kernel._last_sels = fin

    total = np.float32(0.0)
    for core in range(N_CORES):
        b, c = divmod(core, C)
        ws, hsp, ds = fin[core]
        total += _finish_core(np.asarray(res.results[core]["o_all"], np.float32),
                              S, ws, hsp, ds, mw[b, c], mh[b, c], md[b, c])
    return np.float32(total)
